# revision 1
# baseline (speedup 1.0000x reference)
"""Bahdanau (additive) attention kernel for Trainium2, 8 NeuronCores.

Reference computation (per batch b):
    w1q = query @ W1                         # (T, U)
    w2k = value @ W2                         # (S, U)
    scores[t,s] = sum_u scale[u] * tanh(w1q[t,u] + w2k[s,u])
    attn = softmax(scores, axis=-1)          # (T, S)
    context = attn @ value                   # (T, V)

Sharding: batch (B=8) data-parallel, one batch per core. W1/W2/scale replicated.

Per-core design (ACT engine is the roofline: 8.4M tanh elements / 128 lanes
@ 1.2 GHz ~= 55us):
  - u (UNITS=256) lives on SBUF partitions (2 halves of 128).
  - For each t, DVE broadcast-adds w1qT[:,t] over w2kT via tensor_scalar
    (fp32 2x mode); results land in wide buffers so ACT evaluates tanh in
    large chunks (amortizes the per-instruction overhead). Chunk sizes ramp
    up at the start and down at the end to minimize exposed fill/drain.
  - The scale-weighted reduction over u rides the tensor engine in float32r
    (1 cycle/row at moving>=256): stationary operand is a 128x128 matrix G,
    zero except column 63 = scale-half; the slice G[:, 63-t : 127-t] puts
    scale in column t, so each matmul accumulates scale.T @ tanh_tile into
    row t of a (64, 512) PSUM scores tile.
  - Softmax along the free axis with no max-subtraction (|scores| <=
    ||scale||_1 ~ 13, so exp stays well inside fp32 range); the row sum
    comes free via the activation accumulator. The context matmul uses PE
    transposes of the unnormalized exp (batched into one PSUM tile); the
    1/sum normalization is folded into the evacuating scale ops.
  - Input DMAs are split across the two HWDGE queues (sync + scalar);
    value arrives as column pairs and W1/W2 as u-halves so the uh=0
    pipeline start is decoupled from non-critical bytes.
"""

import numpy as np

import concourse.tile as tile
from concourse import bacc, mybir
from concourse.bass_utils import run_bass_kernel_spmd
from concourse.masks import make_identity

B, T, S = 8, 64, 512
QU, VU, U = 1024, 512, 256
N_CORES = 8
F32 = mybir.dt.float32
F32R = mybir.dt.float32r
AF = mybir.ActivationFunctionType
AX = mybir.AxisListType
OP = mybir.AluOpType

# t-chunk sizes for the tanh pipeline: ramp up (uh=0), ramp down (uh=1).
# Steady-state 12 amortizes the ACT per-instruction overhead while keeping
# the PE idle gap per chunk (~2.7us warm) below the ~3.4us HAM re-throttle
# window, so PE stays at 2.4GHz.
CHUNKS_UP = [2, 2, 4, 8] + [12] * 4
CHUNKS_DOWN = [12] * 4 + [8, 4, 3, 1]
MAXC = max(CHUNKS_UP)


def build_program():
    nc = bacc.Bacc(
        "TRN2",
        target_bir_lowering=False,
        debug=False,
        enable_asserts=False,
        num_devices=N_CORES,
    )
    q_d = nc.dram_tensor("query", (T, QU), F32, kind="ExternalInput").ap()
    v_d = nc.dram_tensor("value", (S, VU), F32, kind="ExternalInput").ap()
    w1_d = nc.dram_tensor("w1", (QU, U), F32, kind="ExternalInput").ap()
    w2_d = nc.dram_tensor("w2", (VU, U), F32, kind="ExternalInput").ap()
    sc_d = nc.dram_tensor("scale", (U, 1), F32, kind="ExternalInput").ap()
    ctx_d = nc.dram_tensor("context", (T, VU), F32, kind="ExternalOutput").ap()
    att_d = nc.dram_tensor("attn", (T, S), F32, kind="ExternalOutput").ap()

    with tile.TileContext(nc) as tc:
        with (
            tc.tile_pool(name="const", bufs=1) as cpool,
            tc.tile_pool(name="addp", bufs=3) as addp,
            tc.tile_pool(name="tanhp", bufs=3) as tanhp,
            tc.tile_pool(name="smx", bufs=1) as smxp,
            tc.tile_pool(name="ps_tr", bufs=3, space="PSUM") as ps_tr,
            tc.tile_pool(name="ps_proj", bufs=1, space="PSUM") as ps_proj,
            tc.tile_pool(name="ps_sc", bufs=1, space="PSUM") as ps_sc,
            tc.tile_pool(name="ps_ctx", bufs=1, space="PSUM") as ps_ctx,
        ):
            # ---- input loads, split across the two HWDGE queues ----
            # All transfers are chunked so the (serialized) DMA engines
            # interleave them; the w2kT-gating tensors (value, W2) come first.
            # scalar queue: value in two column-pair DMAs, so each pair's PE
            # transposes and w2kT accumulation matmuls overlap the DMA window
            v_sb = cpool.tile([128, 4 * VU], F32, tag="v")
            for dp in range(2):
                nc.scalar.dma_start(
                    out=v_sb.rearrange("p (c v) -> p c v", c=4)[
                        :, :, 256 * dp : 256 * (dp + 1)
                    ],
                    in_=v_d.rearrange("(c p) v -> c p v", p=128).rearrange(
                        "c p v -> p c v"
                    )[:, :, 256 * dp : 256 * (dp + 1)],
                )
            # sync queue: query, then W2/W1 split by u-half — the first tanh
            # chunk needs only the uh=0 half of the projections, so the
            # uh=1 column halves are deferred behind everything critical.
            q_sb = cpool.tile([T, QU], F32, tag="q")
            nc.sync.dma_start(out=q_sb, in_=q_d)
            sc_stage = cpool.tile([128, 2], F32, tag="sc_stage")
            nc.sync.dma_start(
                out=sc_stage.rearrange("p c -> p c ()"),
                in_=sc_d.rearrange("(c p) x -> p c x", p=128),
            )
            w2_sb = cpool.tile([128, 4 * U], F32, tag="w2")
            w1_sb = cpool.tile([128, 8 * U], F32, tag="w1")
            for uh in range(2):
                nc.sync.dma_start(
                    out=w2_sb.rearrange("p (c u) -> p c u", c=4)[
                        :, :, 128 * uh : 128 * (uh + 1)
                    ],
                    in_=w2_d.rearrange("(c p) u -> c p u", p=128).rearrange(
                        "c p u -> p c u"
                    )[:, :, 128 * uh : 128 * (uh + 1)],
                )
                nc.sync.dma_start(
                    out=w1_sb.rearrange("p (c u) -> p c u", c=8)[
                        :, :, 128 * uh : 128 * (uh + 1)
                    ],
                    in_=w1_d.rearrange("(c p) u -> c p u", p=128).rearrange(
                        "c p u -> p c u"
                    )[:, :, 128 * uh : 128 * (uh + 1)],
                )

            # ---- constants ----
            ident = cpool.tile([128, 128], F32, tag="ident")
            make_identity(nc, ident)
            G = []
            for uh in range(2):
                gs = cpool.tile([128, 128], F32, tag=f"Gs{uh}", name=f"Gs{uh}")
                nc.vector.memset(gs, 0.0)
                nc.vector.tensor_copy(gs[:, 63:64], sc_stage[:, uh : uh + 1])
                g = cpool.tile([128, 128], F32R, tag=f"G{uh}", name=f"G{uh}")
                nc.vector.tensor_copy(g, gs)
                G.append(g)

            # ---- query^T via PE transpose, batched into one PSUM tile:
            # qT_sb[:, 64c:64(c+1)] = query[:, 128c:128(c+1)].T
            ptq = ps_tr.tile([128, 512], F32, tag="tr", name="ptq")
            for c in range(8):
                nc.tensor.transpose(
                    ptq[:, T * c : T * (c + 1)],
                    q_sb[:, 128 * c : 128 * (c + 1)],
                    ident[0:T, 0:T],
                )
            qT_sb = cpool.tile([128, 512], F32, tag="qT")
            nc.scalar.copy(qT_sb, ptq)
            qT = [qT_sb[:, T * c : T * (c + 1)] for c in range(8)]

            # ---- value^T via PE transpose: vt_sb[d] (128v, 512s) ----
            vt_sb = [
                cpool.tile([128, S], F32R, tag=f"vt_{d}", name=f"vt_{d}")
                for d in range(4)
            ]
            for dp in range(2):
                for dd in range(2):
                    d = 2 * dp + dd
                    ptv = ps_tr.tile([128, 512], F32, tag="tr", name=f"ptv_{d}")
                    for c in range(4):
                        nc.tensor.transpose(
                            ptv[:, 128 * c : 128 * (c + 1)],
                            v_sb[:, 512 * c + 128 * d : 512 * c + 128 * (d + 1)],
                            ident,
                        )
                    if dd == 0:
                        nc.vector.tensor_copy(vt_sb[d], ptv)
                    else:
                        nc.scalar.copy(vt_sb[d], ptv)

            # ---- fp32r-rounded copies of DMA'd matmul operands ----
            # (on the otherwise-idle GpSimd engine; DVE is the #2 engine.
            # w2_r per u-half so the uh=0 half doesn't wait on the deferred
            # uh=1 W2 DMA.)
            w2_r = cpool.tile([128, 4 * U], F32R, tag="w2r")
            for uh in range(2):
                nc.gpsimd.tensor_copy(
                    w2_r.rearrange("p (c u) -> p c u", c=4)[
                        :, :, 128 * uh : 128 * (uh + 1)
                    ],
                    w2_sb.rearrange("p (c u) -> p c u", c=4)[
                        :, :, 128 * uh : 128 * (uh + 1)
                    ],
                )
            v_r = cpool.tile([128, 4 * VU], F32R, tag="vr")
            nc.gpsimd.tensor_copy(v_r, v_sb)

            # ---- projections (per u-half) + score cube ----
            # uh=1 projection work is emitted AFTER the uh=0 score chunks so
            # its PSUM-evacuation copies get lower scheduler priority and
            # fill DVE slack instead of stalling the ramp-up adds.
            w1qT = [None, None]
            w2kT = [None, None]
            w2kT_ps = [None, None]

            def emit_proj(uh):
                ps1 = ps_proj.tile([128, T], F32, tag="pj1", name=f"psw1q_{uh}")
                for c in range(8):
                    nc.tensor.matmul(
                        ps1,
                        lhsT=w1_sb[:, 256 * c + 128 * uh : 256 * c + 128 * (uh + 1)],
                        rhs=qT[c],
                        start=(c == 0),
                        stop=(c == 7),
                    )
                st1 = cpool.tile([128, T], F32, tag=f"w1qT_{uh}", name=f"w1qT_{uh}")
                nc.vector.tensor_copy(st1, ps1)
                w1qT[uh] = st1

                ps2 = ps_proj.tile([128, S], F32, tag="pj2", name=f"psw2k_{uh}")
                w2kT_ps[uh] = ps2
                for c in range(4):
                    nc.tensor.matmul(
                        ps2,
                        lhsT=w2_r[:, 256 * c + 128 * uh : 256 * c + 128 * (uh + 1)],
                        rhs=vt_sb[c],
                        start=(c == 0),
                        stop=(c == 3),
                    )
                st2 = cpool.tile([128, S], F32, tag=f"w2kT_{uh}", name=f"w2kT_{uh}")
                nc.vector.tensor_copy(st2, ps2)
                w2kT[uh] = st2

            scores_ps = ps_sc.tile([T, S], F32, tag="scores")
            n_mm = 0

            def emit_chunks(uh):
                nonlocal n_mm
                chunks = CHUNKS_UP if uh == 0 else CHUNKS_DOWN
                t0 = 0
                for ci, csz in enumerate(chunks):
                    tb = tanhp.tile([128, MAXC * S], F32R, tag="tb", name=f"tb_{uh}_{ci}")
                    if uh == 0 and ci < 2:
                        # Fused add+tanh via the per-partition bias, reading
                        # w2k straight from the projection PSUM: skips the
                        # w2kT copy and the DVE adds on the critical path.
                        for j in range(csz):
                            t = t0 + j
                            nc.scalar.activation(
                                tb[:, j * S : (j + 1) * S],
                                w2kT_ps[0],
                                AF.Tanh,
                                bias=w1qT[uh][:, t : t + 1],
                            )
                    else:
                        addb = addp.tile(
                            [128, MAXC * S], F32, tag="addb", name=f"addb_{uh}_{ci}"
                        )
                        for j in range(csz):
                            t = t0 + j
                            nc.vector.tensor_scalar_add(
                                addb[:, j * S : (j + 1) * S],
                                w2kT[uh],
                                w1qT[uh][:, t : t + 1],
                            )
                        nc.scalar.activation(
                            tb[:, 0 : csz * S], addb[:, 0 : csz * S], AF.Tanh
                        )
                    for j in range(csz):
                        t = t0 + j
                        nc.tensor.matmul(
                            scores_ps,
                            lhsT=G[uh][:, 63 - t : 127 - t],
                            rhs=tb[:, j * S : (j + 1) * S],
                            start=(n_mm == 0),
                            stop=(n_mm == 2 * T - 1),
                        )
                        n_mm += 1
                    t0 += csz

            emit_proj(0)
            emit_chunks(0)
            emit_proj(1)
            emit_chunks(1)

            # ---- softmax over s (free axis) ----
            # No max-subtraction: |scores| <= ||scale||_1 (~13), exp is safely
            # within fp32 range. The row sum comes free via the activation
            # accumulator, so exp starts right after the last score matmul.
            e_sb = smxp.tile([T, S], F32, tag="e")
            ssum = smxp.tile([T, 1], F32, tag="ssum")
            nc.scalar.activation(e_sb, scores_ps, AF.Exp, accum_out=ssum)
            rsum = smxp.tile([T, 1], F32, tag="rsum")
            nc.vector.reciprocal(rsum, ssum)
            attn_sb = smxp.tile([T, S], F32, tag="attn")
            nc.vector.tensor_scalar_mul(attn_sb, e_sb, rsum)
            nc.sync.dma_start(out=att_d, in_=attn_sb)

            # ---- context = attn @ value (transpose unnormalized e) ----
            ctx_ps = ps_ctx.tile([T, VU], F32, tag="ctx")
            pte = ps_tr.tile([128, 512], F32, tag="tr", name="pte")
            for c in range(4):
                nc.tensor.transpose(
                    pte[:, T * c : T * (c + 1)],
                    e_sb[:, 128 * c : 128 * (c + 1)],
                    ident[0:T, 0:T],
                )
            eT_sb = cpool.tile([128, 4 * T], F32R, tag="eT")
            for c in range(4):
                sl = slice(T * c, T * (c + 1))
                if c % 2 == 0:
                    nc.vector.tensor_copy(eT_sb[:, sl], pte[:, sl])
                else:
                    nc.scalar.copy(eT_sb[:, sl], pte[:, sl])
                nc.tensor.matmul(
                    ctx_ps,
                    lhsT=eT_sb[:, sl],
                    rhs=v_r[:, 512 * c : 512 * (c + 1)],
                    start=(c == 0),
                    stop=(c == 3),
                )
            ctx_sb = smxp.tile([T, VU], F32, tag="ctxsb")
            nc.scalar.mul(ctx_sb, ctx_ps, rsum)
            nc.scalar.dma_start(out=ctx_d, in_=ctx_sb)

    nc.compile()
    return nc


_NC_CACHE = None


def _get_program():
    global _NC_CACHE
    if _NC_CACHE is None:
        _NC_CACHE = build_program()
    return _NC_CACHE


LAST_RESULTS = None


def make_in_maps(query, value, W1, W2, scale):
    w1 = np.ascontiguousarray(W1, dtype=np.float32)
    w2 = np.ascontiguousarray(W2, dtype=np.float32)
    sc = np.ascontiguousarray(scale, dtype=np.float32).reshape(U, 1)
    return [
        {
            "query": np.ascontiguousarray(query[b], dtype=np.float32),
            "value": np.ascontiguousarray(value[b], dtype=np.float32),
            "w1": w1,
            "w2": w2,
            "scale": sc,
        }
        for b in range(B)
    ]


def kernel(query, value, W1, W2, scale):
    global LAST_RESULTS
    nc = _get_program()
    in_maps = make_in_maps(query, value, W1, W2, scale)
    res = run_bass_kernel_spmd(nc, in_maps, core_ids=list(range(N_CORES)))
    LAST_RESULTS = res
    context = np.stack([res.results[b]["context"] for b in range(B)], axis=0)
    attn = np.stack([res.results[b]["attn"] for b in range(B)], axis=0)
    return context.astype(np.float32), attn.astype(np.float32)


def bench_ns(query, value, W1, W2, scale, reps=30):
    """Wall-clock the SPMD executable (jitted once, inputs pre-sharded).

    Returns (min_ns, median_ns) per call: dispatch + 8-core execution,
    excluding H2D of inputs and D2H of outputs.
    """
    import time

    import jax
    from jax.sharding import Mesh, NamedSharding, PartitionSpec
    from jax.experimental.shard_map import shard_map

    from concourse import bass2jax, mybir as mb

    bass2jax.install_neuronx_cc_hook()
    nc = _get_program()
    in_maps = make_in_maps(query, value, W1, W2, scale)

    partition_name = nc.partition_id_tensor.name if nc.partition_id_tensor else None
    in_names, out_names, out_avals, zero_outs = [], [], [], []
    for alloc in nc.m.functions[0].allocations:
        if not isinstance(alloc, mb.MemoryLocationSet):
            continue
        name = alloc.memorylocations[0].name
        if alloc.kind == "ExternalInput":
            if name != partition_name:
                in_names.append(name)
        elif alloc.kind == "ExternalOutput":
            shape = tuple(alloc.tensor_shape)
            dtype = mb.dt.np(alloc.dtype)
            out_avals.append(jax.core.ShapedArray(shape, dtype))
            out_names.append(name)
            zero_outs.append(np.zeros(shape, dtype))
    n_params = len(in_names)
    n_outs = len(out_avals)
    all_in_names = list(in_names) + list(out_names)
    if partition_name is not None:
        all_in_names.append(partition_name)

    def _body(*args):
        operands = list(args)
        if partition_name is not None:
            operands.append(bass2jax.partition_id_tensor())
        return tuple(
            bass2jax._bass_exec_p.bind(
                *operands,
                out_avals=tuple(out_avals),
                in_names=tuple(all_in_names),
                out_names=tuple(out_names),
                lowering_input_output_aliases=(),
                sim_require_finite=True,
                sim_require_nnan=True,
                nc=nc,
            )
        )

    devices = jax.devices()[:N_CORES]
    mesh = Mesh(np.asarray(devices), ("core",))
    donate = tuple(range(n_params, n_params + n_outs))
    sharded = jax.jit(
        shard_map(
            _body,
            mesh=mesh,
            in_specs=(PartitionSpec("core"),) * (n_params + n_outs),
            out_specs=(PartitionSpec("core"),) * n_outs,
            check_rep=False,
        ),
        donate_argnums=donate,
        keep_unused=True,
    )
    spec = NamedSharding(mesh, PartitionSpec("core"))
    concat_in = [
        jax.device_put(
            np.concatenate([np.asarray(in_maps[c][nm]) for c in range(N_CORES)], 0),
            spec,
        )
        for nm in in_names
    ]
    jax.block_until_ready(concat_in)

    def fresh_zeros():
        zs = [
            jax.device_put(np.zeros((N_CORES * z.shape[0], *z.shape[1:]), z.dtype), spec)
            for z in zero_outs
        ]
        jax.block_until_ready(zs)
        return zs

    out = sharded(*concat_in, *fresh_zeros())  # warm-up / compile
    jax.block_until_ready(out)

    times = []
    for _ in range(reps):
        zs = fresh_zeros()
        t0 = time.perf_counter()
        out = sharded(*concat_in, *zs)
        jax.block_until_ready(out)
        times.append((time.perf_counter() - t0) * 1e9)
    times.sort()
    return times[0], times[len(times) // 2]

